# revision 6
# baseline (speedup 1.0000x reference)
"""Trainium2 Bass kernel for nn_BasicBlock_72928544686679.

Computation (see the reference):
    s  = sign(x)                       # binary activation forward value
    bw = sign(w)                       # binary weights  (w in [0, 0.001) -> ~all ones)
    y' = conv2d(s, bw, pad=1)          # saturating conv: clip at +-2^31 never
                                       # binds (|acc| <= 2304), so it's a plain conv.
    y  = y' * scale[c],  scale = mean|w| over (cin,kh,kw)
    out = BN_trainmode(y) * gamma + beta + x

Sharding: data-parallel over batch B=16 -> 2 images per core on 8 cores.
BN statistics need the full batch, so each core computes per-channel partial
sums (sum y', sum y'^2); a tiny AllGather + local reduce combines them.
The collective is split per cout-half so the first half's gather, BN-constant
math, and normalization all overlap the second half's conv matmuls. A warm-up
collective at kernel start absorbs communicator init + cross-core launch skew.

All sign values are exactly representable in bf16 and PSUM accumulates fp32,
so the conv results are exact integers == the reference f32 conv.
"""

import numpy as np

B = 16
NCORES = 8
IMG = 2            # images per core
C = 256            # Cin == Cout
H = W = 28
P = 128
CT = 2             # Cout tiles of 128
CIN_T = 2          # Cin tiles of 128
KPOS = 9           # 3x3 positions
HP, WP = 30, 32    # padded image rows / row stride (28+2 pad, 32 for alignment)
LH = 14            # output rows per L-half
N_HALF = LH * W    # 392, matmul free dim (one PSUM bank)
EPS = 1e-5
NLOC = float(IMG * H * W)   # 1568  elements per channel per core
NTOT = float(B * H * W)     # 12544 elements per channel globally

_NC_CACHE = {}
LAST_RESULTS = None  # BassKernelResults of the most recent run (for profiling)


def _build_nc():
    import concourse.mybir as mybir
    import concourse.tile as tile
    from concourse import bacc
    from concourse.bass import _add_dep_helper

    f32 = mybir.dt.float32
    bf16 = mybir.dt.bfloat16
    AX = mybir.AxisListType
    OP = mybir.AluOpType
    AF = mybir.ActivationFunctionType

    # Bacc (not plain Bass): its compile() runs generate_event_semaphores,
    # which splits multi-wait instructions to satisfy TRN2's 1-wait limit.
    nc = bacc.Bacc("TRN2", target_bir_lowering=False, num_devices=NCORES)

    xs = nc.dram_tensor("xs", [IMG, C, HP, WP], f32, kind="ExternalInput")
    wt = nc.dram_tensor("wt", [C, KPOS * C], bf16, kind="ExternalInput")  # [cin, pos*C+cout]
    wn = nc.dram_tensor("wn", [C, KPOS * C], f32, kind="ExternalInput")   # [cout, k]
    gm = nc.dram_tensor("gamma", [C], f32, kind="ExternalInput")
    bt = nc.dram_tensor("beta", [C], f32, kind="ExternalInput")
    out = nc.dram_tensor("out", [IMG, C, H, W], f32, kind="ExternalOutput")

    with tile.TileContext(nc) as tc:
        with (
            tc.tile_pool(name="big", bufs=1) as big,
            tc.tile_pool(name="small", bufs=1) as small,
            tc.tile_pool(name="dram", bufs=1, space="DRAM") as dram,
            tc.tile_pool(name="psum", bufs=4, space="PSUM") as psum,
        ):
            # ---- warm-up collective: pays communicator-init and aligns the
            # 8 cores while DMA/sign/conv run, so the real gathers are cheap.
            warm_in = dram.tile([P, 2], f32, tag="warm_in", name="warm_in")
            warm_out = dram.tile([NCORES, P, 2], f32, tag="warm_out",
                                 name="warm_out", addr_space="Shared")
            warm_cc = nc.gpsimd.collective_compute(
                "AllGather", OP.bypass,
                replica_groups=[list(range(NCORES))],
                ins=[warm_in.opt()], outs=[warm_out.opt()],
            )

            # ---- tiles ----
            wt_sb = [big.tile([P, KPOS * C], bf16, tag=f"wt{t}", name=f"wt{t}")
                     for t in range(CIN_T)]
            wsgn = [big.tile([P, KPOS * C], bf16, tag=f"wsgn{t}", name=f"wsgn{t}")
                    for t in range(CIN_T)]
            xpad = [[big.tile([P, HP, WP], f32, tag=f"xp{img}{t}", name=f"xp{img}{t}")
                     for t in range(CIN_T)] for img in range(IMG)]
            xsgn = [[big.tile([P, HP, WP], bf16, tag=f"xg{img}{t}", name=f"xg{img}{t}")
                     for t in range(CIN_T)] for img in range(IMG)]

            # loads on two HWDGE rings: weights on SP, images on ACT
            nc.sync.dma_start(wt_sb[0], wt[0:P, :])
            nc.scalar.dma_start(xpad[0][0], xs[0, 0:P])
            nc.sync.dma_start(wt_sb[1], wt[P:2 * P, :])
            nc.scalar.dma_start(xpad[0][1], xs[0, P:2 * P])
            nc.scalar.dma_start(xpad[1][0], xs[1, 0:P])
            nc.scalar.dma_start(xpad[1][1], xs[1, P:2 * P])

            # weight signs on ScalarE (exact Sign LUT)
            nc.scalar.sign(wsgn[0], wt_sb[0])
            nc.scalar.sign(wsgn[1], wt_sb[1])
            # x signs on VectorE via clamp trick: sign(v) = max(min(v*1e35, 1), -1)
            # (exact for |v| > 1e-31; sign(0)=0 keeps the zero padding)
            for img in range(IMG):
                for t in range(CIN_T):
                    xg = xsgn[img][t]
                    nc.vector.tensor_scalar(xg, xpad[img][t], 1e35, 1.0,
                                            OP.mult, OP.min)
                    nc.vector.tensor_scalar_max(xg, xg, -1.0)

            # ---- non-critical loads via SWDGE: |w| scaling, gamma, beta ----
            wn_sb = []
            for t in range(CIN_T):
                wv = big.tile([P, KPOS * C], f32, tag=f"wn{t}", name=f"wn{t}")
                nc.gpsimd.dma_start(wv, wn[t * P:(t + 1) * P, :])
                wn_sb.append(wv)
            s_sb = small.tile([P, CT], f32, tag="s_sb", name="s_sb")
            for t in range(CT):
                nc.vector.tensor_reduce(
                    out=s_sb[:, t:t + 1], in_=wn_sb[t], axis=AX.X, op=OP.add,
                    apply_absolute_value=True,
                )
            nc.vector.tensor_scalar_mul(s_sb, s_sb, 1.0 / (KPOS * C))

            gm_sb = small.tile([P, CT], f32, tag="gm_sb", name="gm_sb")
            nc.gpsimd.dma_start(gm_sb, gm[:].rearrange("(t p) -> p t", p=P))
            bt_sb = small.tile([P, CT], f32, tag="bt_sb", name="bt_sb")
            nc.gpsimd.dma_start(bt_sb, bt[:].rearrange("(t p) -> p t", p=P))
            # precomputed products used by the post-gather constant math
            ss_sb = small.tile([P, CT], f32, tag="ss_sb", name="ss_sb")  # s^2
            nc.vector.tensor_tensor(ss_sb, s_sb, s_sb, OP.mult)
            sg_sb = small.tile([P, CT], f32, tag="sg_sb", name="sg_sb")  # s*gamma
            nc.vector.tensor_tensor(sg_sb, s_sb, gm_sb, OP.mult)

            ysb = [[big.tile([P, H * W], f32, tag=f"y{img}{ct}", name=f"y{img}{ct}")
                    for ct in range(CT)] for img in range(IMG)]
            A_sb = small.tile([P, CT], f32, tag="A_sb", name="A_sb")
            B_sb = small.tile([P, CT], f32, tag="B_sb", name="B_sb")

            prev_cc = warm_cc
            for ct in range(CT):
                # ---- conv for this cout half: 4 psum groups of 18 matmuls ----
                stats = small.tile([P, IMG * 2, 6], f32, tag=f"st{ct}",
                                   name=f"st{ct}")
                for img in range(IMG):
                    for lh in range(2):
                        ps = psum.tile([P, N_HALF], f32, tag="ps", name="ps")
                        k = 0
                        for t in range(CIN_T):
                            for kh in range(3):
                                for kw in range(3):
                                    rhs = xsgn[img][t][
                                        :, lh * LH + kh: lh * LH + kh + LH, kw: kw + W
                                    ]
                                    pos = kh * 3 + kw
                                    lhsT = wsgn[t][:, pos * C + ct * P:
                                                   pos * C + ct * P + P]
                                    nc.tensor.matmul(
                                        ps, lhsT, rhs, start=(k == 0), stop=(k == 17)
                                    )
                                    k += 1
                        yslice = ysb[img][ct][:, lh * N_HALF:(lh + 1) * N_HALF]
                        nc.scalar.copy(yslice, ps)  # evict raw conv ints to SBUF
                        nc.vector.bn_stats(stats[:, img * 2 + lh, :], yslice)

                # ---- local (sum, sumsq) of y' for this half ----
                mv = small.tile([P, 2], f32, tag=f"mv{ct}", name=f"mv{ct}")
                nc.vector.bn_aggr(mv, stats)
                sums = small.tile([P, 2], f32, tag=f"sums{ct}", name=f"sums{ct}")
                nc.vector.tensor_scalar_mul(sums[:, 0:1], mv[:, 0:1], NLOC)
                msq = small.tile([P, 1], f32, tag=f"msq{ct}", name=f"msq{ct}")
                nc.vector.tensor_tensor(msq, mv[:, 0:1], mv[:, 0:1], OP.mult)
                nc.vector.tensor_add(msq, msq, mv[:, 1:2])
                nc.vector.tensor_scalar_mul(sums[:, 1:2], msq, NLOC)

                # ---- AllGather the 1 KiB of partial sums, reduce locally ----
                ag_in = dram.tile([P, 2], f32, tag=f"ag_in{ct}", name=f"ag_in{ct}")
                ag_out = dram.tile([NCORES, P, 2], f32, tag=f"ag_out{ct}",
                                   name=f"ag_out{ct}", addr_space="Shared")
                nc.sync.dma_start(ag_in[:, :], sums[:, :])
                cc = nc.gpsimd.collective_compute(
                    "AllGather", OP.bypass,
                    replica_groups=[list(range(NCORES))],
                    ins=[ag_in.opt()], outs=[ag_out.opt()],
                )
                _add_dep_helper(cc.ins, prev_cc.ins, sync=True,
                                reason="collective ordering")
                prev_cc = cc
                parts = small.tile([P, 2, NCORES], f32, tag=f"parts{ct}",
                                   name=f"parts{ct}")
                nc.sync.dma_start(parts, ag_out.rearrange("r p c -> p c r"))
                tot = small.tile([P, 2], f32, tag=f"tot{ct}", name=f"tot{ct}")
                nc.vector.tensor_reduce(out=tot, in_=parts, axis=AX.X, op=OP.add)

                # ---- fold scaling + BN + gamma/beta into per-channel affine
                # mean' = S1/n ; var' = S2/n - mean'^2   (stats of raw conv y')
                # v = var' * s^2 + eps ; inv = rsqrt(v)  (Newton-refined)
                # A = s*gamma*inv ; B = beta - mean' * A
                mq = small.tile([P, 2], f32, tag=f"mq{ct}", name=f"mq{ct}")
                nc.vector.tensor_scalar_mul(mq, tot, 1.0 / NTOT)
                vv = small.tile([P, 1], f32, tag=f"vv{ct}", name=f"vv{ct}")
                nc.vector.tensor_tensor(vv, mq[:, 0:1], mq[:, 0:1], OP.mult)
                nc.vector.tensor_tensor(vv, mq[:, 1:2], vv, OP.subtract)
                nc.vector.tensor_scalar(vv, vv, ss_sb[:, ct:ct + 1], EPS,
                                        OP.mult, OP.add)
                sq = small.tile([P, 1], f32, tag=f"sq{ct}", name=f"sq{ct}")
                nc.scalar.sqrt(sq, vv)
                r0 = small.tile([P, 1], f32, tag=f"r0{ct}", name=f"r0{ct}")
                nc.vector.reciprocal(r0, sq)
                e = small.tile([P, 1], f32, tag=f"e{ct}", name=f"e{ct}")
                nc.vector.tensor_tensor(e, vv, r0, OP.mult)
                nc.vector.tensor_tensor(e, e, r0, OP.mult)
                nc.vector.tensor_scalar(e, e, -0.5, 1.5, OP.mult, OP.add)
                nc.vector.tensor_tensor(r0, r0, e, OP.mult)  # inv (refined)
                nc.vector.tensor_tensor(A_sb[:, ct:ct + 1], sg_sb[:, ct:ct + 1],
                                        r0, OP.mult)
                bb = small.tile([P, 1], f32, tag=f"bb{ct}", name=f"bb{ct}")
                nc.vector.tensor_tensor(bb, mq[:, 0:1], A_sb[:, ct:ct + 1], OP.mult)
                nc.vector.tensor_tensor(B_sb[:, ct:ct + 1], bt_sb[:, ct:ct + 1],
                                        bb, OP.subtract)

                # ---- apply affine + residual, write out this half ----
                for img in range(IMG):
                    yo = big.tile([P, H, W], f32, tag=f"yo{img}{ct}",
                                  name=f"yo{img}{ct}")
                    nc.scalar.activation(
                        yo,
                        ysb[img][ct].rearrange("p (a b) -> p a b", b=W),
                        AF.Identity,
                        bias=B_sb[:, ct:ct + 1],
                        scale=A_sb[:, ct:ct + 1],
                    )
                    nc.vector.tensor_add(yo, yo, xpad[img][ct][:, 1:H + 1, 1:W + 1])
                    nc.scalar.dma_start(out[img, ct * P:(ct + 1) * P], yo)

    return nc


def _get_nc():
    if "nc" not in _NC_CACHE:
        nc = _build_nc()
        nc.finalize()  # Bacc defers register allocation to finalize()
        _NC_CACHE["nc"] = nc
    return _NC_CACHE["nc"]


def kernel(**inputs) -> np.ndarray:
    global LAST_RESULTS
    import ml_dtypes

    x = np.ascontiguousarray(np.asarray(inputs["x"], dtype=np.float32))
    w = np.asarray(inputs["weights"], dtype=np.float32)
    gamma = np.ascontiguousarray(np.asarray(inputs["gamma"], dtype=np.float32))
    beta = np.ascontiguousarray(np.asarray(inputs["beta"], dtype=np.float32))

    # host-side layout glue: zero-pad x to 30x32 rows, pre-transpose weights.
    # wt only feeds sign() on-device, so the bf16 cast is sign-preserving.
    xp = np.zeros((B, C, HP, WP), np.float32)
    xp[:, :, 1:H + 1, 1:W + 1] = x
    wt = np.ascontiguousarray(
        w.transpose(1, 2, 3, 0).reshape(C, KPOS * C)   # [cin, (kh*3+kw)*C + cout]
    ).astype(ml_dtypes.bfloat16)
    wn = np.ascontiguousarray(w.reshape(C, KPOS * C))  # [cout, cin*9 + kh*3 + kw]

    nc = _get_nc()
    from concourse.bass_utils import run_bass_kernel_spmd

    in_maps = [
        {
            "xs": np.ascontiguousarray(xp[IMG * c: IMG * (c + 1)]),
            "wt": wt,
            "wn": wn,
            "gamma": gamma,
            "beta": beta,
        }
        for c in range(NCORES)
    ]
    res = run_bass_kernel_spmd(nc, in_maps, core_ids=list(range(NCORES)))
    LAST_RESULTS = res
    return np.concatenate([res.results[c]["out"] for c in range(NCORES)], axis=0)


# revision 7
# speedup vs baseline: 1.0138x; 1.0138x over previous
"""Trainium2 Bass kernel for nn_BasicBlock_72928544686679.

Computation (see the reference):
    s  = sign(x)                       # binary activation forward value
    bw = sign(w)                       # binary weights  (w in [0, 0.001) -> ~all ones)
    y' = conv2d(s, bw, pad=1)          # saturating conv: clip at +-2^31 never
                                       # binds (|acc| <= 2304), so it's a plain conv.
    y  = y' * scale[c],  scale = mean|w| over (cin,kh,kw)
    out = BN_trainmode(y) * gamma + beta + x

Sharding: data-parallel over batch B=16 -> 2 images per core on 8 cores.
BN statistics need the full batch, so each core computes per-channel partial
sums (sum y', sum y'^2); a tiny AllGather + local reduce combines them.
The collective is split per cout-half so the first half's gather, BN-constant
math, and normalization all overlap the second half's conv matmuls. A warm-up
collective at kernel start absorbs communicator init + cross-core launch skew.

All sign values are exactly representable in bf16 and PSUM accumulates fp32,
so the conv results are exact integers == the reference f32 conv.
"""

import numpy as np

B = 16
NCORES = 8
IMG = 2            # images per core
C = 256            # Cin == Cout
H = W = 28
P = 128
CT = 2             # Cout tiles of 128
CIN_T = 2          # Cin tiles of 128
KPOS = 9           # 3x3 positions
HP, WP = 30, 32    # padded image rows / row stride (28+2 pad, 32 for alignment)
LH = 14            # output rows per L-half
N_HALF = LH * W    # 392, matmul free dim (one PSUM bank)
EPS = 1e-5
NLOC = float(IMG * H * W)   # 1568  elements per channel per core
NTOT = float(B * H * W)     # 12544 elements per channel globally

_NC_CACHE = {}
LAST_RESULTS = None  # BassKernelResults of the most recent run (for profiling)


def _build_nc():
    import concourse.mybir as mybir
    import concourse.tile as tile
    from concourse import bacc
    from concourse.bass import _add_dep_helper

    f32 = mybir.dt.float32
    bf16 = mybir.dt.bfloat16
    AX = mybir.AxisListType
    OP = mybir.AluOpType
    AF = mybir.ActivationFunctionType

    # Bacc (not plain Bass): its compile() runs generate_event_semaphores,
    # which splits multi-wait instructions to satisfy TRN2's 1-wait limit.
    nc = bacc.Bacc("TRN2", target_bir_lowering=False, num_devices=NCORES)

    xs = nc.dram_tensor("xs", [IMG, C, HP, WP], f32, kind="ExternalInput")
    wt = nc.dram_tensor("wt", [C, KPOS * C], bf16, kind="ExternalInput")  # [cin, pos*C+cout]
    wn = nc.dram_tensor("wn", [C, KPOS * C], f32, kind="ExternalInput")   # [cout, k]
    gm = nc.dram_tensor("gamma", [C], f32, kind="ExternalInput")
    bt = nc.dram_tensor("beta", [C], f32, kind="ExternalInput")
    out = nc.dram_tensor("out", [IMG, C, H, W], f32, kind="ExternalOutput")

    with tile.TileContext(nc) as tc:
        with (
            tc.tile_pool(name="big", bufs=1) as big,
            tc.tile_pool(name="small", bufs=1) as small,
            tc.tile_pool(name="dram", bufs=1, space="DRAM") as dram,
            tc.tile_pool(name="psum", bufs=4, space="PSUM") as psum,
        ):
            # ---- warm-up collective: pays communicator-init and aligns the
            # 8 cores while DMA/sign/conv run, so the real gathers are cheap.
            warm_in = dram.tile([P, 2], f32, tag="warm_in", name="warm_in")
            warm_out = dram.tile([NCORES, P, 2], f32, tag="warm_out",
                                 name="warm_out", addr_space="Shared")
            warm_cc = nc.gpsimd.collective_compute(
                "AllGather", OP.bypass,
                replica_groups=[list(range(NCORES))],
                ins=[warm_in.opt()], outs=[warm_out.opt()],
            )

            # ---- tiles ----
            wt_sb = [big.tile([P, KPOS * C], bf16, tag=f"wt{t}", name=f"wt{t}")
                     for t in range(CIN_T)]
            wsgn = [big.tile([P, KPOS * C], bf16, tag=f"wsgn{t}", name=f"wsgn{t}")
                    for t in range(CIN_T)]
            xpad = [[big.tile([P, HP, WP], f32, tag=f"xp{img}{t}", name=f"xp{img}{t}")
                     for t in range(CIN_T)] for img in range(IMG)]
            xsgn = [[big.tile([P, HP, WP], bf16, tag=f"xg{img}{t}", name=f"xg{img}{t}")
                     for t in range(CIN_T)] for img in range(IMG)]

            # loads on two HWDGE rings: weights on SP, images on ACT
            nc.sync.dma_start(wt_sb[0], wt[0:P, :])
            nc.scalar.dma_start(xpad[0][0], xs[0, 0:P])
            nc.sync.dma_start(wt_sb[1], wt[P:2 * P, :])
            nc.scalar.dma_start(xpad[0][1], xs[0, P:2 * P])
            nc.scalar.dma_start(xpad[1][0], xs[1, 0:P])
            nc.scalar.dma_start(xpad[1][1], xs[1, P:2 * P])

            # weight signs on ScalarE (exact Sign LUT)
            nc.scalar.sign(wsgn[0], wt_sb[0])
            nc.scalar.sign(wsgn[1], wt_sb[1])
            # x signs on VectorE via clamp trick: sign(v) = max(min(v*1e35, 1), -1)
            # (exact for |v| > 1e-31; sign(0)=0 keeps the zero padding)
            for img in range(IMG):
                for t in range(CIN_T):
                    xg = xsgn[img][t]
                    nc.vector.tensor_scalar(xg, xpad[img][t], 1e35, 1.0,
                                            OP.mult, OP.min)
                    nc.vector.tensor_scalar_max(xg, xg, -1.0)

            # ---- non-critical loads via SWDGE: |w| scaling, gamma, beta ----
            wn_sb = []
            for t in range(CIN_T):
                wv = big.tile([P, KPOS * C], f32, tag=f"wn{t}", name=f"wn{t}")
                nc.gpsimd.dma_start(wv, wn[t * P:(t + 1) * P, :])
                wn_sb.append(wv)
            s_sb = small.tile([P, CT], f32, tag="s_sb", name="s_sb")
            for t in range(CT):
                nc.vector.tensor_reduce(
                    out=s_sb[:, t:t + 1], in_=wn_sb[t], axis=AX.X, op=OP.add,
                    apply_absolute_value=True,
                )
            nc.vector.tensor_scalar_mul(s_sb, s_sb, 1.0 / (KPOS * C))

            gm_sb = small.tile([P, CT], f32, tag="gm_sb", name="gm_sb")
            nc.gpsimd.dma_start(gm_sb, gm[:].rearrange("(t p) -> p t", p=P))
            bt_sb = small.tile([P, CT], f32, tag="bt_sb", name="bt_sb")
            nc.gpsimd.dma_start(bt_sb, bt[:].rearrange("(t p) -> p t", p=P))
            # precomputed products used by the post-gather constant math
            ss_sb = small.tile([P, CT], f32, tag="ss_sb", name="ss_sb")  # s^2
            nc.vector.tensor_tensor(ss_sb, s_sb, s_sb, OP.mult)
            sg_sb = small.tile([P, CT], f32, tag="sg_sb", name="sg_sb")  # s*gamma
            nc.vector.tensor_tensor(sg_sb, s_sb, gm_sb, OP.mult)

            ysb = [[big.tile([P, H * W], f32, tag=f"y{img}{ct}", name=f"y{img}{ct}")
                    for ct in range(CT)] for img in range(IMG)]

            # ---- conv: per (cout_tile, img, l_half) accumulate 18 matmuls ----
            stats = [small.tile([P, IMG * 2, 6], f32, tag=f"st{ct}", name=f"st{ct}")
                     for ct in range(CT)]
            for ct in range(CT):
                for img in range(IMG):
                    for lh in range(2):
                        ps = psum.tile([P, N_HALF], f32, tag="ps", name="ps")
                        k = 0
                        for t in range(CIN_T):
                            for kh in range(3):
                                for kw in range(3):
                                    rhs = xsgn[img][t][
                                        :, lh * LH + kh: lh * LH + kh + LH, kw: kw + W
                                    ]
                                    pos = kh * 3 + kw
                                    lhsT = wsgn[t][:, pos * C + ct * P:
                                                   pos * C + ct * P + P]
                                    nc.tensor.matmul(
                                        ps, lhsT, rhs, start=(k == 0), stop=(k == 17)
                                    )
                                    k += 1
                        yslice = ysb[img][ct][:, lh * N_HALF:(lh + 1) * N_HALF]
                        nc.scalar.copy(yslice, ps)  # evict raw conv ints to SBUF
                        nc.vector.bn_stats(stats[ct][:, img * 2 + lh, :], yslice)

            # ---- local (sum, sumsq) of y' per channel ----
            sums = small.tile([P, CT, 2], f32, tag="sums", name="sums")
            for ct in range(CT):
                mv = small.tile([P, 2], f32, tag=f"mv{ct}", name=f"mv{ct}")
                nc.vector.bn_aggr(mv, stats[ct])
                nc.vector.tensor_scalar_mul(sums[:, ct, 0:1], mv[:, 0:1], NLOC)
                msq = small.tile([P, 1], f32, tag=f"msq{ct}", name=f"msq{ct}")
                nc.vector.tensor_tensor(msq, mv[:, 0:1], mv[:, 0:1], OP.mult)
                nc.vector.tensor_add(msq, msq, mv[:, 1:2])
                nc.vector.tensor_scalar_mul(sums[:, ct, 1:2], msq, NLOC)

            # ---- AllGather the 2 KiB of partial sums, reduce locally ----
            ag_in = dram.tile([P, CT * 2], f32, tag="ag_in", name="ag_in")
            ag_out = dram.tile([NCORES, P, CT * 2], f32, tag="ag_out",
                               name="ag_out", addr_space="Shared")
            nc.sync.dma_start(ag_in[:, :], sums[:, :, :])
            cc = nc.gpsimd.collective_compute(
                "AllGather", OP.bypass,
                replica_groups=[list(range(NCORES))],
                ins=[ag_in.opt()], outs=[ag_out.opt()],
            )
            _add_dep_helper(cc.ins, warm_cc.ins, sync=True,
                            reason="collective warm-up ordering")
            parts = small.tile([P, CT * 2, NCORES], f32, tag="parts", name="parts")
            nc.sync.dma_start(parts, ag_out.rearrange("r p c -> p c r"))
            tot = small.tile([P, CT, 2], f32, tag="tot", name="tot")
            nc.vector.tensor_reduce(out=tot.rearrange("p a b -> p (a b)"),
                                    in_=parts, axis=AX.X, op=OP.add)

            # ---- fold scaling + BN + gamma/beta into per-channel affine ----
            # mean' = S1/n ; var' = S2/n - mean'^2   (stats of raw conv y')
            # v = var' * s^2 + eps ; inv = rsqrt(v)  (Newton-refined)
            # A = s*gamma*inv ; B = beta - mean' * A
            A_sb = small.tile([P, CT], f32, tag="A_sb", name="A_sb")
            B_sb = small.tile([P, CT], f32, tag="B_sb", name="B_sb")
            mp = small.tile([P, CT], f32, tag="mp", name="mp")
            nc.vector.tensor_scalar_mul(mp, tot[:, :, 0], 1.0 / NTOT)
            vv = small.tile([P, CT], f32, tag="vv", name="vv")
            nc.vector.tensor_scalar_mul(vv, tot[:, :, 1], 1.0 / NTOT)
            t2 = small.tile([P, CT], f32, tag="t2", name="t2")
            nc.vector.tensor_tensor(t2, mp, mp, OP.mult)
            nc.vector.tensor_tensor(vv, vv, t2, OP.subtract)      # var'
            nc.vector.tensor_tensor(vv, vv, ss_sb, OP.mult)
            nc.vector.tensor_scalar_add(vv, vv, EPS)              # v
            sq = small.tile([P, CT], f32, tag="sq", name="sq")
            nc.scalar.sqrt(sq, vv)
            r0 = small.tile([P, CT], f32, tag="r0", name="r0")
            nc.vector.reciprocal(r0, sq)
            nc.vector.tensor_tensor(t2, vv, r0, OP.mult)
            nc.vector.tensor_tensor(t2, t2, r0, OP.mult)
            nc.vector.tensor_scalar(t2, t2, -0.5, 1.5, OP.mult, OP.add)
            nc.vector.tensor_tensor(r0, r0, t2, OP.mult)          # inv (refined)
            nc.vector.tensor_tensor(A_sb, sg_sb, r0, OP.mult)
            nc.vector.tensor_tensor(B_sb, mp, A_sb, OP.mult)
            nc.vector.tensor_tensor(B_sb, bt_sb, B_sb, OP.subtract)

            # ---- apply affine + residual, write out ----
            for img in range(IMG):
                for ct in range(CT):
                    yo = big.tile([P, H, W], f32, tag=f"yo{img}{ct}",
                                  name=f"yo{img}{ct}")
                    nc.scalar.activation(
                        yo,
                        ysb[img][ct].rearrange("p (a b) -> p a b", b=W),
                        AF.Identity,
                        bias=B_sb[:, ct:ct + 1],
                        scale=A_sb[:, ct:ct + 1],
                    )
                    nc.vector.tensor_add(yo, yo, xpad[img][ct][:, 1:H + 1, 1:W + 1])
                    nc.scalar.dma_start(out[img, ct * P:(ct + 1) * P], yo)

    return nc


def _get_nc():
    if "nc" not in _NC_CACHE:
        nc = _build_nc()
        nc.finalize()  # Bacc defers register allocation to finalize()
        _NC_CACHE["nc"] = nc
    return _NC_CACHE["nc"]


def kernel(**inputs) -> np.ndarray:
    global LAST_RESULTS
    import ml_dtypes

    x = np.ascontiguousarray(np.asarray(inputs["x"], dtype=np.float32))
    w = np.asarray(inputs["weights"], dtype=np.float32)
    gamma = np.ascontiguousarray(np.asarray(inputs["gamma"], dtype=np.float32))
    beta = np.ascontiguousarray(np.asarray(inputs["beta"], dtype=np.float32))

    # host-side layout glue: zero-pad x to 30x32 rows, pre-transpose weights.
    # wt only feeds sign() on-device, so the bf16 cast is sign-preserving.
    xp = np.zeros((B, C, HP, WP), np.float32)
    xp[:, :, 1:H + 1, 1:W + 1] = x
    wt = np.ascontiguousarray(
        w.transpose(1, 2, 3, 0).reshape(C, KPOS * C)   # [cin, (kh*3+kw)*C + cout]
    ).astype(ml_dtypes.bfloat16)
    wn = np.ascontiguousarray(w.reshape(C, KPOS * C))  # [cout, cin*9 + kh*3 + kw]

    nc = _get_nc()
    from concourse.bass_utils import run_bass_kernel_spmd

    in_maps = [
        {
            "xs": np.ascontiguousarray(xp[IMG * c: IMG * (c + 1)]),
            "wt": wt,
            "wn": wn,
            "gamma": gamma,
            "beta": beta,
        }
        for c in range(NCORES)
    ]
    res = run_bass_kernel_spmd(nc, in_maps, core_ids=list(range(NCORES)))
    LAST_RESULTS = res
    return np.concatenate([res.results[c]["out"] for c in range(NCORES)], axis=0)


# revision 11
# speedup vs baseline: 1.0852x; 1.0704x over previous
"""Trainium2 Bass kernel for nn_BasicBlock_72928544686679.

Computation (see the reference):
    s  = sign(x)                       # binary activation forward value
    bw = sign(w)                       # binary weights  (w in [0, 0.001) -> ~all ones)
    y' = conv2d(s, bw, pad=1)          # saturating conv: clip at +-2^31 never
                                       # binds (|acc| <= 2304), so it's a plain conv.
    y  = y' * scale[c],  scale = mean|w| over (cin,kh,kw)
    out = BN_trainmode(y) * gamma + beta + x

Sharding: data-parallel over batch B=16 -> 2 images per core on 8 cores.
BN statistics need the full batch, so each core computes per-channel partial
sums (sum y', sum y'^2); a tiny AllGather + local reduce combines them.
The collective is split per cout-half so the first half's gather, BN-constant
math, and normalization all overlap the second half's conv matmuls. A warm-up
collective at kernel start absorbs communicator init + cross-core launch skew.

All sign values are exactly representable in bf16 and PSUM accumulates fp32,
so the conv results are exact integers == the reference f32 conv.
"""

import numpy as np

B = 16
NCORES = 8
IMG = 2            # images per core
C = 256            # Cin == Cout
H = W = 28
P = 128
CT = 2             # Cout tiles of 128
CIN_T = 2          # Cin tiles of 128
KPOS = 9           # 3x3 positions
HP, WP = 30, 32    # padded image rows / row stride (28+2 pad, 32 for alignment)
LH = 14            # output rows per L-half
N_HALF = LH * W    # 392, matmul free dim (one PSUM bank)
EPS = 1e-5
NLOC = float(IMG * H * W)   # 1568  elements per channel per core
NTOT = float(B * H * W)     # 12544 elements per channel globally

_NC_CACHE = {}
LAST_RESULTS = None  # BassKernelResults of the most recent run (for profiling)


def _build_nc():
    import concourse.mybir as mybir
    import concourse.tile as tile
    from concourse import bacc
    from concourse.bass import _add_dep_helper

    f32 = mybir.dt.float32
    bf16 = mybir.dt.bfloat16
    AX = mybir.AxisListType
    OP = mybir.AluOpType
    AF = mybir.ActivationFunctionType

    # Bacc (not plain Bass): its compile() runs generate_event_semaphores,
    # which splits multi-wait instructions to satisfy TRN2's 1-wait limit.
    nc = bacc.Bacc("TRN2", target_bir_lowering=False, num_devices=NCORES)

    xs = nc.dram_tensor("xs", [IMG, C, HP, WP], f32, kind="ExternalInput")
    wt = nc.dram_tensor("wt", [C, KPOS * C], bf16, kind="ExternalInput")  # [cin, pos*C+cout]
    wn = nc.dram_tensor("wn", [C, KPOS * C], f32, kind="ExternalInput")   # [cout, k]
    gm = nc.dram_tensor("gamma", [C], f32, kind="ExternalInput")
    bt = nc.dram_tensor("beta", [C], f32, kind="ExternalInput")
    out = nc.dram_tensor("out", [IMG, C, H, W], f32, kind="ExternalOutput")

    with tile.TileContext(nc) as tc:
        with (
            tc.tile_pool(name="big", bufs=1) as big,
            tc.tile_pool(name="small", bufs=1) as small,
            tc.tile_pool(name="dram", bufs=1, space="DRAM") as dram,
            tc.tile_pool(name="psum", bufs=4, space="PSUM") as psum,
        ):
            # ---- warm-up collective: pays communicator-init and aligns the
            # 8 cores while DMA/sign/conv run, so the real gathers are cheap.
            warm_in = dram.tile([P, 2], f32, tag="warm_in", name="warm_in")
            warm_out = dram.tile([NCORES, P, 2], f32, tag="warm_out",
                                 name="warm_out", addr_space="Shared")
            warm_cc = nc.gpsimd.collective_compute(
                "AllGather", OP.bypass,
                replica_groups=[list(range(NCORES))],
                ins=[warm_in.opt()], outs=[warm_out.opt()],
            )

            # ---- tiles ----
            wt_sb = [big.tile([P, KPOS * C], bf16, tag=f"wt{t}", name=f"wt{t}")
                     for t in range(CIN_T)]
            wsgn = [big.tile([P, KPOS * C], bf16, tag=f"wsgn{t}", name=f"wsgn{t}")
                    for t in range(CIN_T)]
            xpad = [[big.tile([P, HP, WP], f32, tag=f"xp{img}{t}", name=f"xp{img}{t}")
                     for t in range(CIN_T)] for img in range(IMG)]
            xsgn = [[big.tile([P, HP, WP], bf16, tag=f"xg{img}{t}", name=f"xg{img}{t}")
                     for t in range(CIN_T)] for img in range(IMG)]

            # loads split across both HWDGE rings, critical-path first:
            # weight halves (gate all matmuls), then images in use order
            HK = KPOS * C // 2
            nc.sync.dma_start(wt_sb[0][:, 0:HK], wt[0:P, 0:HK])
            nc.scalar.dma_start(wt_sb[0][:, HK:], wt[0:P, HK:])
            nc.sync.dma_start(wt_sb[1][:, 0:HK], wt[P:2 * P, 0:HK])
            nc.scalar.dma_start(wt_sb[1][:, HK:], wt[P:2 * P, HK:])
            nc.sync.dma_start(xpad[0][0], xs[0, 0:P])
            nc.scalar.dma_start(xpad[0][1], xs[0, P:2 * P])
            nc.sync.dma_start(xpad[1][0], xs[1, 0:P])
            nc.scalar.dma_start(xpad[1][1], xs[1, P:2 * P])

            # weight signs on ScalarE (exact Sign LUT)
            nc.scalar.sign(wsgn[0], wt_sb[0])
            nc.scalar.sign(wsgn[1], wt_sb[1])
            # x signs on VectorE via clamp trick: sign(v) = max(min(v*1e35, 1), -1)
            # (exact for |v| > 1e-31; sign(0)=0 keeps the zero padding)
            for img in range(IMG):
                for t in range(CIN_T):
                    xg = xsgn[img][t]
                    nc.vector.tensor_scalar(xg, xpad[img][t], 1e35, 1.0,
                                            OP.mult, OP.min)
                    nc.vector.tensor_scalar_max(xg, xg, -1.0)

            # ---- non-critical loads via SWDGE: |w| scaling, gamma, beta.
            # Deferred until conv is underway so they don't steal HBM
            # bandwidth from the critical wt/x loads (dep added below).
            wn_sb = []
            wn_dmas = []
            for t in range(CIN_T):
                wv = big.tile([P, KPOS * C], f32, tag=f"wn{t}", name=f"wn{t}")
                wn_dmas.append(nc.gpsimd.dma_start(wv, wn[t * P:(t + 1) * P, :]))
                wn_sb.append(wv)
            s_sb = small.tile([P, CT], f32, tag="s_sb", name="s_sb")
            for t in range(CT):
                nc.vector.tensor_reduce(
                    out=s_sb[:, t:t + 1], in_=wn_sb[t], axis=AX.X, op=OP.add,
                    apply_absolute_value=True,
                )
            nc.vector.tensor_scalar_mul(s_sb, s_sb, 1.0 / (KPOS * C))

            gm_sb = small.tile([P, CT], f32, tag="gm_sb", name="gm_sb")
            nc.gpsimd.dma_start(gm_sb, gm[:].rearrange("(t p) -> p t", p=P))
            bt_sb = small.tile([P, CT], f32, tag="bt_sb", name="bt_sb")
            nc.gpsimd.dma_start(bt_sb, bt[:].rearrange("(t p) -> p t", p=P))
            # precomputed products used by the post-gather constant math
            ss_sb = small.tile([P, CT], f32, tag="ss_sb", name="ss_sb")  # s^2
            nc.vector.tensor_tensor(ss_sb, s_sb, s_sb, OP.mult)
            sg_sb = small.tile([P, CT], f32, tag="sg_sb", name="sg_sb")  # s*gamma
            nc.vector.tensor_tensor(sg_sb, s_sb, gm_sb, OP.mult)

            ysb = [[big.tile([P, H * W], f32, tag=f"y{img}{ct}", name=f"y{img}{ct}")
                    for ct in range(CT)] for img in range(IMG)]

            # ---- conv: per (cout_tile, img, l_half) accumulate 18 matmuls ----
            stats = [small.tile([P, IMG * 2, 6], f32, tag=f"st{ct}", name=f"st{ct}")
                     for ct in range(CT)]
            first_evict = None
            for ct in range(CT):
                for img in range(IMG):
                    for lh in range(2):
                        ps = psum.tile([P, N_HALF], f32, tag="ps", name="ps")
                        k = 0
                        for t in range(CIN_T):
                            for kh in range(3):
                                for kw in range(3):
                                    rhs = xsgn[img][t][
                                        :, lh * LH + kh: lh * LH + kh + LH, kw: kw + W
                                    ]
                                    pos = kh * 3 + kw
                                    lhsT = wsgn[t][:, pos * C + ct * P:
                                                   pos * C + ct * P + P]
                                    nc.tensor.matmul(
                                        ps, lhsT, rhs, start=(k == 0), stop=(k == 17)
                                    )
                                    k += 1
                        yslice = ysb[img][ct][:, lh * N_HALF:(lh + 1) * N_HALF]
                        ev = nc.scalar.copy(yslice, ps)  # evict conv ints to SBUF
                        if first_evict is None:
                            first_evict = ev
                        nc.vector.bn_stats(stats[ct][:, img * 2 + lh, :], yslice)

            # hold the bulky wn loads back until conv is underway
            for dma in wn_dmas:
                _add_dep_helper(dma.ins, first_evict.ins, sync=True,
                                reason="defer wn load off the startup HBM window")

            # ---- local (sum, sumsq) of y' per channel ----
            sums = small.tile([P, CT, 2], f32, tag="sums", name="sums")
            for ct in range(CT):
                mv = small.tile([P, 2], f32, tag=f"mv{ct}", name=f"mv{ct}")
                nc.vector.bn_aggr(mv, stats[ct])
                nc.vector.tensor_scalar_mul(sums[:, ct, 0:1], mv[:, 0:1], NLOC)
                msq = small.tile([P, 1], f32, tag=f"msq{ct}", name=f"msq{ct}")
                nc.vector.tensor_tensor(msq, mv[:, 0:1], mv[:, 0:1], OP.mult)
                nc.vector.tensor_add(msq, msq, mv[:, 1:2])
                nc.vector.tensor_scalar_mul(sums[:, ct, 1:2], msq, NLOC)

            # ---- AllGather the 2 KiB of partial sums, reduce locally ----
            ag_in = dram.tile([P, CT * 2], f32, tag="ag_in", name="ag_in")
            ag_out = dram.tile([NCORES, P, CT * 2], f32, tag="ag_out",
                               name="ag_out", addr_space="Shared")
            nc.sync.dma_start(ag_in[:, :], sums[:, :, :])
            cc = nc.gpsimd.collective_compute(
                "AllGather", OP.bypass,
                replica_groups=[list(range(NCORES))],
                ins=[ag_in.opt()], outs=[ag_out.opt()],
            )
            _add_dep_helper(cc.ins, warm_cc.ins, sync=True,
                            reason="collective warm-up ordering")
            parts = small.tile([P, CT * 2, NCORES], f32, tag="parts", name="parts")
            nc.sync.dma_start(parts, ag_out.rearrange("r p c -> p c r"))
            tot = small.tile([P, CT, 2], f32, tag="tot", name="tot")
            nc.vector.tensor_reduce(out=tot.rearrange("p a b -> p (a b)"),
                                    in_=parts, axis=AX.X, op=OP.add)

            # ---- fold scaling + BN + gamma/beta into per-channel affine ----
            # mean' = S1/n ; var' = S2/n - mean'^2   (stats of raw conv y')
            # v = var' * s^2 + eps ; inv = rsqrt(v)  (Newton-refined)
            # A = s*gamma*inv ; B = beta - mean' * A
            A_sb = small.tile([P, CT], f32, tag="A_sb", name="A_sb")
            B_sb = small.tile([P, CT], f32, tag="B_sb", name="B_sb")
            mp = small.tile([P, CT], f32, tag="mp", name="mp")
            nc.vector.tensor_scalar_mul(mp, tot[:, :, 0], 1.0 / NTOT)
            vv = small.tile([P, CT], f32, tag="vv", name="vv")
            nc.vector.tensor_scalar_mul(vv, tot[:, :, 1], 1.0 / NTOT)
            t2 = small.tile([P, CT], f32, tag="t2", name="t2")
            nc.vector.tensor_tensor(t2, mp, mp, OP.mult)
            nc.vector.tensor_tensor(vv, vv, t2, OP.subtract)      # var'
            nc.vector.tensor_tensor(vv, vv, ss_sb, OP.mult)
            nc.vector.tensor_scalar_add(vv, vv, EPS)              # v
            sq = small.tile([P, CT], f32, tag="sq", name="sq")
            nc.scalar.sqrt(sq, vv)
            r0 = small.tile([P, CT], f32, tag="r0", name="r0")
            nc.vector.reciprocal(r0, sq)
            nc.vector.tensor_tensor(t2, vv, r0, OP.mult)
            nc.vector.tensor_tensor(t2, t2, r0, OP.mult)
            nc.vector.tensor_scalar(t2, t2, -0.5, 1.5, OP.mult, OP.add)
            nc.vector.tensor_tensor(r0, r0, t2, OP.mult)          # inv (refined)
            nc.vector.tensor_tensor(A_sb, sg_sb, r0, OP.mult)
            nc.vector.tensor_tensor(B_sb, mp, A_sb, OP.mult)
            nc.vector.tensor_tensor(B_sb, bt_sb, B_sb, OP.subtract)

            # ---- apply affine + residual, write out ----
            for img in range(IMG):
                for ct in range(CT):
                    yo = big.tile([P, H, W], f32, tag=f"yo{img}{ct}",
                                  name=f"yo{img}{ct}")
                    nc.scalar.activation(
                        yo,
                        ysb[img][ct].rearrange("p (a b) -> p a b", b=W),
                        AF.Identity,
                        bias=B_sb[:, ct:ct + 1],
                        scale=A_sb[:, ct:ct + 1],
                    )
                    nc.vector.tensor_add(yo, yo, xpad[img][ct][:, 1:H + 1, 1:W + 1])
                    nc.scalar.dma_start(out[img, ct * P:(ct + 1) * P], yo)

    return nc


def _get_nc():
    if "nc" not in _NC_CACHE:
        nc = _build_nc()
        nc.finalize()  # Bacc defers register allocation to finalize()
        _NC_CACHE["nc"] = nc
    return _NC_CACHE["nc"]


def kernel(**inputs) -> np.ndarray:
    global LAST_RESULTS
    import ml_dtypes

    x = np.ascontiguousarray(np.asarray(inputs["x"], dtype=np.float32))
    w = np.asarray(inputs["weights"], dtype=np.float32)
    gamma = np.ascontiguousarray(np.asarray(inputs["gamma"], dtype=np.float32))
    beta = np.ascontiguousarray(np.asarray(inputs["beta"], dtype=np.float32))

    # host-side layout glue: zero-pad x to 30x32 rows, pre-transpose weights.
    # wt only feeds sign() on-device, so the bf16 cast is sign-preserving.
    xp = np.zeros((B, C, HP, WP), np.float32)
    xp[:, :, 1:H + 1, 1:W + 1] = x
    wt = np.ascontiguousarray(
        w.transpose(1, 2, 3, 0).reshape(C, KPOS * C)   # [cin, (kh*3+kw)*C + cout]
    ).astype(ml_dtypes.bfloat16)
    wn = np.ascontiguousarray(w.reshape(C, KPOS * C))  # [cout, cin*9 + kh*3 + kw]

    nc = _get_nc()
    from concourse.bass_utils import run_bass_kernel_spmd

    in_maps = [
        {
            "xs": np.ascontiguousarray(xp[IMG * c: IMG * (c + 1)]),
            "wt": wt,
            "wn": wn,
            "gamma": gamma,
            "beta": beta,
        }
        for c in range(NCORES)
    ]
    res = run_bass_kernel_spmd(nc, in_maps, core_ids=list(range(NCORES)))
    LAST_RESULTS = res
    return np.concatenate([res.results[c]["out"] for c in range(NCORES)], axis=0)
